# revision 10
# baseline (speedup 1.0000x reference)
"""Causal multi-head attention (B=2, S=2048, D=1024, H=16) on 8 TRN2 NeuronCores.

Sharding (data + tensor parallel, per the hint): core c handles batch b = c//4
and head-group g = c%4 (4 heads = 256 channels). Wq/Wk/Wv are split column-wise
(rows of the torch-layout weight) and Wo row-wise over those channels. Each core
computes a partial output [S, D]; the host sums the 4 group-partials per batch
and adds the bias.

Per-core pipeline (everything in transposed [channel, seq] space so no on-chip
transposes are needed; all matmul operands are bf16 (fp32 PSUM accumulation)):

  xT [D, S] (host-transposed)                          resident in SBUF
  qT/kT [o, S] = (wT-chunk).T @ xT                     o = 4 heads x 64
  v    [S, o]  = (xT-chunk).T @ wvT  (+ ones column)
  scoresT[kv, sq] = kT_h.T @ qT_h      per (128-kv-chunk, 512-sq-block),
                                       causally skipped; pairs share a
                                       2-bank PSUM tile
  p = exp(SCALE * scoresT)             one ACT op per pair, PSUM->SBUF
  causal mask on diagonal chunks       GPSIMD affine_select, fill 0
  ctxT[hd+1, sq] += v_chunk.T @ p      ones column accumulates the softmax
                                       denominator in row hd
  ctxT_norm = ctxT * (1/denom)         DVE; denom broadcast across partitions
                                       by an SBUF->SBUF DMA
  out[s, :] = sum_t ctxT-chunk.T @ woT-chunk           -> DRAM (partial)
"""

import sys

import numpy as np

sys.path.insert(0, "/opt/trn_rl_repo")

B, S, D, H = 2, 2048, 1024, 16
HD = 64
SCALE = 1.0 / float(np.sqrt(HD))
NCORES = 8
GROUPS = NCORES // B      # head-groups per batch (4)
HN = H // GROUPS          # heads per core (4)
O = HN * HD               # channels per core (256)

_CACHE = {}


def emit_mha(tc, out_d, xT_d, wqT_d, wkT_d, wvT_d, woT_d, *, seq, dmodel, hn, hd,
             scale):
    import concourse.mybir as mybir

    nc = tc.nc
    f32 = mybir.dt.float32
    bf16 = mybir.dt.bfloat16
    EXP = mybir.ActivationFunctionType.Exp
    GE = mybir.AluOpType.is_ge

    o = hn * hd                # local qkv channels
    hpt = 128 // hd            # heads per qT/kT partition tile
    nqt = o // 128             # qT/kT partition tiles
    dc = dmodel // 128         # contraction chunks of the model dim
    sqb = min(512, seq)        # sq block = moving free dim of attention matmuls
    nj = seq // sqb            # sq blocks
    kcpb = sqb // 128          # kv chunks per sq block
    nn = (dmodel + 511) // 512 # out-proj free-dim chunks
    nw = dmodel // nn          # out-proj free chunk width

    assert o % 128 == 0 and dmodel % 128 == 0 and seq % sqb == 0
    assert kcpb % 2 == 0, "kv chunks per sq block must pair up"

    with (
        tc.tile_pool(name="persist", bufs=1) as pp,
        tc.tile_pool(name="work", bufs=3) as wp,
        tc.tile_pool(name="psum", bufs=2, space="PSUM") as psp,
        tc.tile_pool(name="dscr", bufs=2, space="DRAM") as dsp,
    ):
        # ---------------- DRAM loads ----------------
        def loadw(d_ap, nm):
            w = pp.tile([128, dc, o], bf16, name=nm, tag=nm)
            nc.sync.dma_start(out=w, in_=d_ap.rearrange("(t p) o -> p t o", p=128))
            return w

        wq = loadw(wqT_d, "wq")
        wk = loadw(wkT_d, "wk")
        wv = loadw(wvT_d, "wv")
        wo = []
        for t in range(nqt):
            wot = pp.tile([128, dmodel], bf16, name=f"wo{t}", tag=f"wo{t}")
            nc.sync.dma_start(out=wot, in_=woT_d[t * 128:(t + 1) * 128, :])
            wo.append(wot)

        xt = [pp.tile([128, seq], bf16, name=f"xt{t}", tag=f"xt{t}")
              for t in range(dc)]
        for j in range(nj):
            sl = slice(j * sqb, (j + 1) * sqb)
            for t in range(dc):
                nc.sync.dma_start(out=xt[t][:, sl],
                                  in_=xT_d[t * 128:(t + 1) * 128, sl])

        # ---------------- Q/K/V projections ----------------
        qt = [pp.tile([128, seq], bf16, name=f"qt{t}", tag=f"qt{t}")
              for t in range(nqt)]
        kt = [pp.tile([128, seq], bf16, name=f"kt{t}", tag=f"kt{t}")
              for t in range(nqt)]
        vt = [pp.tile([128, hn, hd + 1], bf16, name=f"vt{s}", tag=f"vt{s}")
              for s in range(seq // 128)]
        # memset can't target the matmul dtype directly on every ISA; stage
        # the ones in f32 and convert via tensor_copy.
        ones = pp.tile([128, hn], f32, name="ones", tag="ones")
        nc.vector.memset(ones, 1.0)

        # ---------------- pipelined emission ----------------
        # PE is in-order: to keep it dense (and HAM warm) while ACT works
        # through the exps, projection/out-projection matmuls are emitted as
        # "filler" between attention matmuls via generators that yield after
        # each PE instruction.

        ctxt = [pp.tile([128, seq], bf16, name=f"ctxt{t}", tag=f"ctxt{t}")
                for t in range(nqt)]

        def qk_gen(j, ts):
            sl = slice(j * sqb, (j + 1) * sqb)
            for t in ts:
                for wsrc, dst, pn in ((wq, qt, "q"), (wk, kt, "k")):
                    ps = psp.tile([128, sqb], f32,
                                  name=f"ps_f_{pn}{t}_{j}", tag="ps_f")
                    for d in range(dc):
                        nc.tensor.matmul(
                            ps,
                            lhsT=wsrc[:, d, t * 128:(t + 1) * 128],
                            rhs=xt[d][:, sl],
                            start=(d == 0), stop=(d == dc - 1))
                        if d == dc - 1:
                            nc.vector.tensor_copy(out=dst[t][:, sl], in_=ps)
                        yield

        def v_gen(j):
            for sc in range(j * kcpb, (j + 1) * kcpb):
                ps = psp.tile([128, o], f32, name=f"ps_f_v{sc}", tag="ps_f")
                for d in range(dc):
                    nc.tensor.matmul(
                        ps,
                        lhsT=xt[d][:, sc * 128:(sc + 1) * 128],
                        rhs=wv[:, d, :],
                        start=(d == 0), stop=(d == dc - 1))
                    if d == dc - 1:
                        nc.vector.tensor_copy(
                            out=vt[sc][:, :, 0:hd],
                            in_=ps.rearrange("p (h e) -> p h e", h=hn))
                        nc.vector.tensor_copy(
                            out=vt[sc][:, :, hd:hd + 1],
                            in_=ones.rearrange("p (h e) -> p h e", e=1))
                    yield

        def outproj_gen(j):
            for st in range(j * kcpb, (j + 1) * kcpb):
                for n in range(nn):
                    ps = psp.tile([128, nw], f32, name=f"ps_f_o{st}_{n}",
                                  tag="ps_f")
                    for t in range(nqt):
                        nc.tensor.matmul(
                            ps,
                            lhsT=ctxt[t][:, st * 128:(st + 1) * 128],
                            rhs=wo[t][:, n * nw:(n + 1) * nw],
                            start=(t == 0), stop=(t == nqt - 1))
                        if t == nqt - 1:
                            ob = wp.tile([128, nw], f32, name=f"ob{st}_{n}",
                                         tag="ob", bufs=2)
                            nc.vector.tensor_copy(out=ob, in_=ps)
                            nc.sync.dma_start(
                                out=out_d[st * 128:(st + 1) * 128,
                                          n * nw:(n + 1) * nw],
                                in_=ob)
                        yield

        from collections import deque
        filler_q = deque()

        def pump(n=1):
            while n > 0 and filler_q:
                try:
                    next(filler_q[0][1])
                    n -= 1
                except StopIteration:
                    filler_q.popleft()

        def drain(tag):
            while any(t == tag for t, _ in filler_q):
                try:
                    next(filler_q[0][1])
                except StopIteration:
                    filler_q.popleft()

        # block-0 projections must precede attention; later blocks feed the
        # filler queue.
        for _ in qk_gen(0, [0]):
            pass
        for _ in v_gen(0):
            pass
        filler_q.append(("p0b", qk_gen(0, [1])))
        for j in range(1, nj):
            filler_q.append((f"p{j}", qk_gen(j, list(range(nqt)))))
            filler_q.append((f"p{j}", v_gen(j)))

        for j in range(nj):
            sl = slice(j * sqb, (j + 1) * sqb)
            if j > 0:
                drain(f"p{j}")
            for hp in range(nqt):
                if j == 0 and hp == 1:
                    drain("p0b")
                hA, hB = 2 * hp, 2 * hp + 1
                nchunks = (j + 1) * kcpb
                ctx = [psp.tile([128, sqb], f32, name=f"ps_c{j}_{hp}_{x}",
                                tag="ps_c") for x in (0, 1)]

                def scores_chunk(c):
                    # heads hA/hB sit at partition bases 0/64 of the same
                    # qT/kT tile; K=64 matmuls on disjoint row strips run
                    # concurrently in the PE array.
                    ps_s = psp.tile([128, 2 * sqb], f32,
                                    name=f"ps_s{j}_{hp}_{c}", tag="ps_s")
                    pt = wp.tile([128, 2 * sqb], bf16, name=f"pt{j}_{hp}_{c}",
                                 tag="pt")
                    for i, base in ((0, 0), (1, hd)):
                        nc.tensor.matmul(
                            ps_s[:, i * sqb:(i + 1) * sqb],
                            lhsT=kt[hp][base:base + hd,
                                        c * 128:(c + 1) * 128],
                            rhs=qt[hp][base:base + hd, sl],
                            start=True, stop=True)
                    nc.scalar.activation(out=pt, in_=ps_s, func=EXP, scale=scale)
                    if 128 * (c + 1) > j * sqb:  # diagonal chunk
                        for i in (0, 1):
                            nc.gpsimd.affine_select(
                                out=pt[:, i * sqb:(i + 1) * sqb],
                                in_=pt[:, i * sqb:(i + 1) * sqb],
                                compare_op=GE, fill=0.0,
                                base=j * sqb - c * 128,
                                channel_multiplier=-1,
                                pattern=[[1, sqb]])
                    return pt

                def pv_chunk(c, pt, last):
                    for i, h in ((0, hA), (1, hB)):
                        nc.tensor.matmul(
                            ctx[i][0:hd + 1, :],
                            lhsT=vt[c][:, h, :],
                            rhs=pt[:, i * sqb:(i + 1) * sqb],
                            start=(c == 0), stop=last)

                prev = None
                for c in range(nchunks):
                    cur = scores_chunk(c)
                    pump(3 if c == 0 else 1)
                    if prev is not None:
                        pv_chunk(c - 1, prev, last=False)
                        pump(1)
                    prev = cur
                pv_chunk(nchunks - 1, prev, last=True)

                for i, h in ((0, hA), (1, hB)):
                    base = (h % hpt) * hd
                    # Softmax denominators sit in one PSUM partition; a
                    # single-lane DVE reciprocal over 512 elements measures
                    # ~3.4us, so spread them over 128 partitions (via a DRAM
                    # bounce, SBUF APs can't remap partitions), invert, and
                    # bounce back for the partition broadcast.
                    fw = sqb // 128
                    dn = wp.tile([1, sqb], f32, name=f"dn{j}_{h}", tag="rc",
                                 bufs=2)
                    nc.vector.tensor_copy(out=dn, in_=ctx[i][hd:hd + 1, :])
                    dd = dsp.tile([1, sqb], f32, name=f"dd{j}_{h}", tag="dd")
                    nc.sync.dma_start(out=dd, in_=dn)
                    rs = wp.tile([128, fw], f32, name=f"rs{j}_{h}", tag="rs",
                                 bufs=2)
                    nc.sync.dma_start(
                        out=rs, in_=dd.rearrange("o (p f) -> (o p) f", p=128))
                    nc.vector.reciprocal(out=rs, in_=rs)
                    rd = dsp.tile([1, sqb], f32, name=f"rd{j}_{h}", tag="rd")
                    nc.sync.dma_start(
                        out=rd.rearrange("o (p f) -> (o p) f", p=128), in_=rs)
                    rcb = wp.tile([hd, sqb], f32, name=f"rcb{j}_{h}",
                                  tag="rcb", bufs=2)
                    nc.sync.dma_start(out=rcb, in_=rd.to_broadcast((hd, sqb)))
                    nc.vector.tensor_mul(ctxt[hp][base:base + hd, sl],
                                         ctx[i][0:hd, :], rcb)
            filler_q.append((f"o{j}", outproj_gen(j)))
        while filler_q:
            pump(1)


def build_nc(*, seq=S, dmodel=D, hn=HN, hd=HD, scale=SCALE, num_devices=NCORES):
    import concourse.mybir as mybir
    import concourse.tile as tile
    from concourse import bacc

    f32 = mybir.dt.float32
    o = hn * hd
    nc = bacc.Bacc("TRN2", target_bir_lowering=False, debug=False,
                   num_devices=num_devices)
    bf16 = mybir.dt.bfloat16
    xT = nc.dram_tensor("xT", (dmodel, seq), bf16, kind="ExternalInput").ap()
    wqT = nc.dram_tensor("wqT", (dmodel, o), bf16, kind="ExternalInput").ap()
    wkT = nc.dram_tensor("wkT", (dmodel, o), bf16, kind="ExternalInput").ap()
    wvT = nc.dram_tensor("wvT", (dmodel, o), bf16, kind="ExternalInput").ap()
    woT = nc.dram_tensor("woT", (o, dmodel), bf16, kind="ExternalInput").ap()
    out = nc.dram_tensor("out", (seq, dmodel), f32, kind="ExternalOutput").ap()
    with tile.TileContext(nc) as tc:
        emit_mha(tc, out, xT, wqT, wkT, wvT, woT, seq=seq, dmodel=dmodel,
                 hn=hn, hd=hd, scale=scale)
    nc.compile()
    return nc


def make_in_maps(x, Wq, Wk, Wv, Wo):
    import ml_dtypes
    bf16 = ml_dtypes.bfloat16

    def cvt(a):
        return np.ascontiguousarray(np.asarray(a, np.float32)).astype(bf16)

    x = np.asarray(x, np.float32)
    Wq = np.asarray(Wq, np.float32)
    Wk = np.asarray(Wk, np.float32)
    Wv = np.asarray(Wv, np.float32)
    Wo = np.asarray(Wo, np.float32)
    in_maps = []
    for c in range(NCORES):
        b, g = divmod(c, GROUPS)
        ch = slice(g * O, (g + 1) * O)
        in_maps.append({
            "xT": cvt(x[b].T),
            "wqT": cvt(Wq[ch, :].T),
            "wkT": cvt(Wk[ch, :].T),
            "wvT": cvt(Wv[ch, :].T),
            "woT": cvt(Wo[:, ch].T),
        })
    return in_maps


def combine_outputs(parts, bo):
    bo = np.asarray(bo, np.float64)
    out = np.empty((B, S, D), np.float32)
    for b in range(B):
        acc = np.zeros((S, D), np.float64)
        for g in range(GROUPS):
            acc += parts[b * GROUPS + g]
        out[b] = (acc + bo).astype(np.float32)
    return out


def run_on_hw(in_maps, **kwargs):
    from concourse import bass_utils
    if "nc" not in _CACHE:
        _CACHE["nc"] = build_nc()
    return bass_utils.run_bass_kernel_spmd(
        _CACHE["nc"], in_maps, core_ids=list(range(NCORES)), **kwargs)


def kernel(x, Wq, Wk, Wv, Wo, bo):
    res = run_on_hw(make_in_maps(x, Wq, Wk, Wv, Wo))
    parts = [res.results[c]["out"] for c in range(NCORES)]
    return combine_outputs(parts, bo)


# revision 11
# speedup vs baseline: 1.3052x; 1.3052x over previous
"""Causal multi-head attention (B=2, S=2048, D=1024, H=16) on 8 TRN2 NeuronCores.

Sharding (data + tensor parallel, per the hint): core c handles batch b = c//4
and head-group g = c%4 (4 heads = 256 channels). Wq/Wk/Wv are split column-wise
(rows of the torch-layout weight) and Wo row-wise over those channels. Each core
computes a partial output [S, D]; the host sums the 4 group-partials per batch
and adds the bias.

Per-core pipeline (everything in transposed [channel, seq] space so no on-chip
transposes are needed; all matmul operands are bf16 (fp32 PSUM accumulation)):

  xT [D, S] (host-transposed)                          resident in SBUF
  qT/kT [o, S] = (wT-chunk).T @ xT                     o = 4 heads x 64
  v    [S, o]  = (xT-chunk).T @ wvT  (+ ones column)
  scoresT[kv, sq] = kT_h.T @ qT_h      per (128-kv-chunk, 512-sq-block),
                                       causally skipped; pairs share a
                                       2-bank PSUM tile
  p = exp(SCALE * scoresT)             one ACT op per pair, PSUM->SBUF
  causal mask on diagonal chunks       GPSIMD affine_select, fill 0
  ctxT[hd+1, sq] += v_chunk.T @ p      ones column accumulates the softmax
                                       denominator in row hd
  ctxT_norm = ctxT * (1/denom)         DVE; denom broadcast across partitions
                                       by an SBUF->SBUF DMA
  out[s, :] = sum_t ctxT-chunk.T @ woT-chunk           -> DRAM (partial)
"""

import sys

import numpy as np

sys.path.insert(0, "/opt/trn_rl_repo")

B, S, D, H = 2, 2048, 1024, 16
HD = 64
SCALE = 1.0 / float(np.sqrt(HD))
NCORES = 8
GROUPS = NCORES // B      # head-groups per batch (4)
HN = H // GROUPS          # heads per core (4)
O = HN * HD               # channels per core (256)

_CACHE = {}


def emit_mha(tc, out_d, xT_d, wqT_d, wkT_d, wvT_d, woT_d, *, seq, dmodel, hn, hd,
             scale):
    import concourse.mybir as mybir

    nc = tc.nc
    f32 = mybir.dt.float32
    bf16 = mybir.dt.bfloat16
    EXP = mybir.ActivationFunctionType.Exp
    GE = mybir.AluOpType.is_ge

    o = hn * hd                # local qkv channels
    hpt = 128 // hd            # heads per qT/kT partition tile
    nqt = o // 128             # qT/kT partition tiles
    dc = dmodel // 128         # contraction chunks of the model dim
    sqb = min(512, seq)        # sq block = moving free dim of attention matmuls
    nj = seq // sqb            # sq blocks
    kcpb = sqb // 128          # kv chunks per sq block
    nn = (dmodel + 511) // 512 # out-proj free-dim chunks
    nw = dmodel // nn          # out-proj free chunk width

    assert o % 128 == 0 and dmodel % 128 == 0 and seq % sqb == 0
    assert kcpb % 2 == 0, "kv chunks per sq block must pair up"

    with (
        tc.tile_pool(name="persist", bufs=1) as pp,
        tc.tile_pool(name="work", bufs=3) as wp,
        tc.tile_pool(name="psum", bufs=2, space="PSUM") as psp,
        tc.tile_pool(name="dscr", bufs=4, space="DRAM") as dsp,
    ):
        # ---------------- DRAM loads ----------------
        def loadw(d_ap, nm):
            w = pp.tile([128, dc, o], bf16, name=nm, tag=nm)
            nc.sync.dma_start(out=w, in_=d_ap.rearrange("(t p) o -> p t o", p=128))
            return w

        wq = loadw(wqT_d, "wq")
        wk = loadw(wkT_d, "wk")
        wv = loadw(wvT_d, "wv")
        wo = []
        for t in range(nqt):
            wot = pp.tile([128, dmodel], bf16, name=f"wo{t}", tag=f"wo{t}")
            nc.sync.dma_start(out=wot, in_=woT_d[t * 128:(t + 1) * 128, :])
            wo.append(wot)

        xt = [pp.tile([128, seq], bf16, name=f"xt{t}", tag=f"xt{t}")
              for t in range(dc)]
        for j in range(nj):
            sl = slice(j * sqb, (j + 1) * sqb)
            for t in range(dc):
                nc.sync.dma_start(out=xt[t][:, sl],
                                  in_=xT_d[t * 128:(t + 1) * 128, sl])

        # ---------------- Q/K/V projections ----------------
        qt = [pp.tile([128, seq], bf16, name=f"qt{t}", tag=f"qt{t}")
              for t in range(nqt)]
        kt = [pp.tile([128, seq], bf16, name=f"kt{t}", tag=f"kt{t}")
              for t in range(nqt)]
        vt = [pp.tile([128, hn, hd + 1], bf16, name=f"vt{s}", tag=f"vt{s}")
              for s in range(seq // 128)]
        # memset can't target the matmul dtype directly on every ISA; stage
        # the ones in f32 and convert via tensor_copy.
        ones = pp.tile([128, hn], f32, name="ones", tag="ones")
        nc.vector.memset(ones, 1.0)

        # ---------------- pipelined emission ----------------
        # PE is in-order: to keep it dense (and HAM warm) while ACT works
        # through the exps, projection/out-projection matmuls are emitted as
        # "filler" between attention matmuls via generators that yield after
        # each PE instruction.

        ctxt = [pp.tile([128, seq], bf16, name=f"ctxt{t}", tag=f"ctxt{t}")
                for t in range(nqt)]

        def qk_gen(j, ts):
            sl = slice(j * sqb, (j + 1) * sqb)
            for t in ts:
                for wsrc, dst, pn in ((wq, qt, "q"), (wk, kt, "k")):
                    ps = psp.tile([128, sqb], f32,
                                  name=f"ps_f_{pn}{t}_{j}", tag="ps_f")
                    for d in range(dc):
                        nc.tensor.matmul(
                            ps,
                            lhsT=wsrc[:, d, t * 128:(t + 1) * 128],
                            rhs=xt[d][:, sl],
                            start=(d == 0), stop=(d == dc - 1))
                        if d == dc - 1:
                            nc.vector.tensor_copy(out=dst[t][:, sl], in_=ps)
                        yield

        def v_gen(j):
            for sc in range(j * kcpb, (j + 1) * kcpb):
                ps = psp.tile([128, o], f32, name=f"ps_f_v{sc}", tag="ps_f")
                for d in range(dc):
                    nc.tensor.matmul(
                        ps,
                        lhsT=xt[d][:, sc * 128:(sc + 1) * 128],
                        rhs=wv[:, d, :],
                        start=(d == 0), stop=(d == dc - 1))
                    if d == dc - 1:
                        nc.vector.tensor_copy(
                            out=vt[sc][:, :, 0:hd],
                            in_=ps.rearrange("p (h e) -> p h e", h=hn))
                        nc.vector.tensor_copy(
                            out=vt[sc][:, :, hd:hd + 1],
                            in_=ones.rearrange("p (h e) -> p h e", e=1))
                    yield

        def outproj_gen(j):
            for st in range(j * kcpb, (j + 1) * kcpb):
                for n in range(nn):
                    ps = psp.tile([128, nw], f32, name=f"ps_f_o{st}_{n}",
                                  tag="ps_f")
                    for t in range(nqt):
                        nc.tensor.matmul(
                            ps,
                            lhsT=ctxt[t][:, st * 128:(st + 1) * 128],
                            rhs=wo[t][:, n * nw:(n + 1) * nw],
                            start=(t == 0), stop=(t == nqt - 1))
                        if t == nqt - 1:
                            ob = wp.tile([128, nw], f32, name=f"ob{st}_{n}",
                                         tag="ob", bufs=2)
                            nc.vector.tensor_copy(out=ob, in_=ps)
                            nc.sync.dma_start(
                                out=out_d[st * 128:(st + 1) * 128,
                                          n * nw:(n + 1) * nw],
                                in_=ob)
                        yield

        from collections import deque
        filler_q = deque()

        def pump(n=1):
            while n > 0 and filler_q:
                try:
                    next(filler_q[0][1])
                    n -= 1
                except StopIteration:
                    filler_q.popleft()

        def drain(tag):
            while any(t == tag for t, _ in filler_q):
                try:
                    next(filler_q[0][1])
                except StopIteration:
                    filler_q.popleft()

        # block-0 projections must precede attention; later blocks feed the
        # filler queue.
        for _ in qk_gen(0, [0]):
            pass
        for _ in v_gen(0):
            pass
        filler_q.append(("p0b", qk_gen(0, [1])))
        for j in range(1, nj):
            filler_q.append((f"p{j}", qk_gen(j, list(range(nqt)))))
            filler_q.append((f"p{j}", v_gen(j)))

        for j in range(nj):
            sl = slice(j * sqb, (j + 1) * sqb)
            if j > 0:
                drain(f"p{j}")
            for hp in range(nqt):
                if j == 0 and hp == 1:
                    drain("p0b")
                hA, hB = 2 * hp, 2 * hp + 1
                nchunks = (j + 1) * kcpb
                ctx = [psp.tile([128, sqb], f32, name=f"ps_c{j}_{hp}_{x}",
                                tag="ps_c") for x in (0, 1)]

                def scores_chunk(c):
                    # heads hA/hB sit at partition bases 0/64 of the same
                    # qT/kT tile; K=64 matmuls on disjoint row strips run
                    # concurrently in the PE array.
                    ps_s = psp.tile([128, 2 * sqb], f32,
                                    name=f"ps_s{j}_{hp}_{c}", tag="ps_s")
                    pt = wp.tile([128, 2 * sqb], bf16, name=f"pt{j}_{hp}_{c}",
                                 tag="pt")
                    for i, base in ((0, 0), (1, hd)):
                        nc.tensor.matmul(
                            ps_s[:, i * sqb:(i + 1) * sqb],
                            lhsT=kt[hp][base:base + hd,
                                        c * 128:(c + 1) * 128],
                            rhs=qt[hp][base:base + hd, sl],
                            start=True, stop=True)
                    nc.scalar.activation(out=pt, in_=ps_s, func=EXP, scale=scale)
                    if 128 * (c + 1) > j * sqb:  # diagonal chunk
                        for i in (0, 1):
                            nc.gpsimd.affine_select(
                                out=pt[:, i * sqb:(i + 1) * sqb],
                                in_=pt[:, i * sqb:(i + 1) * sqb],
                                compare_op=GE, fill=0.0,
                                base=j * sqb - c * 128,
                                channel_multiplier=-1,
                                pattern=[[1, sqb]])
                    return pt

                def pv_chunk(c, pt, last):
                    for i, h in ((0, hA), (1, hB)):
                        nc.tensor.matmul(
                            ctx[i][0:hd + 1, :],
                            lhsT=vt[c][:, h, :],
                            rhs=pt[:, i * sqb:(i + 1) * sqb],
                            start=(c == 0), stop=last)

                prev = None
                for c in range(nchunks):
                    cur = scores_chunk(c)
                    pump(3 if c == 0 else 1)
                    if prev is not None:
                        pv_chunk(c - 1, prev, last=False)
                        pump(1)
                    prev = cur
                pv_chunk(nchunks - 1, prev, last=True)

                for i, h in ((0, hA), (1, hB)):
                    base = (h % hpt) * hd
                    # Evacuate the accumulated (unnormalized) context + its
                    # denominator row to SBUF right away: this frees the ctx
                    # PSUM slot in one copy, so the next head-pair's PV
                    # matmuls never wait on the (long) normalization chain.
                    cu = wp.tile([hd + 1, sqb], f32, name=f"cu{j}_{h}",
                                 tag="cu", bufs=4)
                    nc.vector.tensor_copy(out=cu, in_=ctx[i][0:hd + 1, :])
                    # Softmax denominators sit in one partition; a
                    # single-lane DVE reciprocal over 512 elements measures
                    # ~3.4us, so spread them over 128 partitions (via a DRAM
                    # bounce, SBUF APs can't remap partitions), invert, and
                    # bounce back for the partition broadcast.
                    dd = dsp.tile([1, sqb], f32, name=f"dd{j}_{h}", tag="dd")
                    nc.sync.dma_start(out=dd, in_=cu[hd:hd + 1, :])
                    rs = wp.tile([128, sqb // 128], f32, name=f"rs{j}_{h}",
                                 tag="rs", bufs=4)
                    nc.sync.dma_start(
                        out=rs, in_=dd.rearrange("o (p f) -> (o p) f", p=128))
                    nc.vector.reciprocal(out=rs, in_=rs)
                    rd = dsp.tile([1, sqb], f32, name=f"rd{j}_{h}", tag="rd")
                    nc.sync.dma_start(
                        out=rd.rearrange("o (p f) -> (o p) f", p=128), in_=rs)
                    rcb = wp.tile([hd, sqb], f32, name=f"rcb{j}_{h}",
                                  tag="rcb", bufs=4)
                    nc.sync.dma_start(out=rcb, in_=rd.to_broadcast((hd, sqb)))
                    nc.vector.tensor_mul(ctxt[hp][base:base + hd, sl],
                                         cu[0:hd, :], rcb)
            filler_q.append((f"o{j}", outproj_gen(j)))
        while filler_q:
            pump(1)


def build_nc(*, seq=S, dmodel=D, hn=HN, hd=HD, scale=SCALE, num_devices=NCORES):
    import concourse.mybir as mybir
    import concourse.tile as tile
    from concourse import bacc

    f32 = mybir.dt.float32
    o = hn * hd
    nc = bacc.Bacc("TRN2", target_bir_lowering=False, debug=False,
                   num_devices=num_devices)
    bf16 = mybir.dt.bfloat16
    xT = nc.dram_tensor("xT", (dmodel, seq), bf16, kind="ExternalInput").ap()
    wqT = nc.dram_tensor("wqT", (dmodel, o), bf16, kind="ExternalInput").ap()
    wkT = nc.dram_tensor("wkT", (dmodel, o), bf16, kind="ExternalInput").ap()
    wvT = nc.dram_tensor("wvT", (dmodel, o), bf16, kind="ExternalInput").ap()
    woT = nc.dram_tensor("woT", (o, dmodel), bf16, kind="ExternalInput").ap()
    out = nc.dram_tensor("out", (seq, dmodel), f32, kind="ExternalOutput").ap()
    with tile.TileContext(nc) as tc:
        emit_mha(tc, out, xT, wqT, wkT, wvT, woT, seq=seq, dmodel=dmodel,
                 hn=hn, hd=hd, scale=scale)
    nc.compile()
    return nc


def make_in_maps(x, Wq, Wk, Wv, Wo):
    import ml_dtypes
    bf16 = ml_dtypes.bfloat16

    def cvt(a):
        return np.ascontiguousarray(np.asarray(a, np.float32)).astype(bf16)

    x = np.asarray(x, np.float32)
    Wq = np.asarray(Wq, np.float32)
    Wk = np.asarray(Wk, np.float32)
    Wv = np.asarray(Wv, np.float32)
    Wo = np.asarray(Wo, np.float32)
    in_maps = []
    for c in range(NCORES):
        b, g = divmod(c, GROUPS)
        ch = slice(g * O, (g + 1) * O)
        in_maps.append({
            "xT": cvt(x[b].T),
            "wqT": cvt(Wq[ch, :].T),
            "wkT": cvt(Wk[ch, :].T),
            "wvT": cvt(Wv[ch, :].T),
            "woT": cvt(Wo[:, ch].T),
        })
    return in_maps


def combine_outputs(parts, bo):
    bo = np.asarray(bo, np.float64)
    out = np.empty((B, S, D), np.float32)
    for b in range(B):
        acc = np.zeros((S, D), np.float64)
        for g in range(GROUPS):
            acc += parts[b * GROUPS + g]
        out[b] = (acc + bo).astype(np.float32)
    return out


def run_on_hw(in_maps, **kwargs):
    from concourse import bass_utils
    if "nc" not in _CACHE:
        _CACHE["nc"] = build_nc()
    return bass_utils.run_bass_kernel_spmd(
        _CACHE["nc"], in_maps, core_ids=list(range(NCORES)), **kwargs)


def kernel(x, Wq, Wk, Wv, Wo, bo):
    res = run_on_hw(make_in_maps(x, Wq, Wk, Wv, Wo))
    parts = [res.results[c]["out"] for c in range(NCORES)]
    return combine_outputs(parts, bo)
